# revision 8
# baseline (speedup 1.0000x reference)
"""DLRM (embedding_lookup) Trainium2 Bass kernel.

Strategy: pure data parallelism over the batch. Each of the 8 NeuronCores
holds all 26 embedding tables (replicated in its HBM, flattened to one
[26*200000, 64] tensor so indirect DMA sees base offset 0) and processes
a 512-sample slice of the 4096 batch end-to-end:

  bottom MLP (13->512->256->64) feature-major on PE/ACT
  embedding gathers: per (table, sample-tile, lookup) one
      gpsimd.indirect_dma_start with a [128,1] int32 offset column
      (host pre-biases indices by t*V) fetching 128 rows, one per
      partition, into staging [128, 20, 64]; a DVE add-tree pools the
      20 lookups; PE transposes pooled [128,64] -> [64,128] into the
      feature-major feat matrix [1792, 512] (64 zero pad rows)
  top MLP (1728->512->256->1) + sigmoid on PE/ACT

No collectives; host shards inputs / concatenates outputs. This is
perfectly load balanced (26 tables over 8 cores would be 4-vs-3
imbalanced table-wise) and needs no all-to-all.
"""

import numpy as np

import concourse.bass as bass
import concourse.mybir as mybir
import concourse.tile as tile
from concourse import bacc
from concourse.bass_utils import run_bass_kernel_spmd
from concourse.masks import make_identity

F32 = mybir.dt.float32
I32 = mybir.dt.int32
AF = mybir.ActivationFunctionType

B = 4096
T = 26
V = 200000
L = 20
D = 64
DENSE = 13
NCORES = 8


def build_bass(T=T, V=V, NT=4, L=L, D=D):
    Bc = NT * 128
    NCH = (T + 2) // 2
    pad_rows = NCH * 128 - D * (T + 1)
    assert Bc <= 512

    nc = bacc.Bacc(
        "TRN2", target_bir_lowering=False, debug=False,
        enable_asserts=False, num_devices=1,
    )

    tables = nc.dram_tensor("tables", [T * V, D], mybir.dt.float16, kind="ExternalInput")
    idx = nc.dram_tensor("idx", [128, T * NT * L], I32, kind="ExternalInput")
    xdt = nc.dram_tensor("xdt", [128, Bc], F32, kind="ExternalInput")
    wb0 = nc.dram_tensor("wb0", [128, 512], F32, kind="ExternalInput")
    wb1 = nc.dram_tensor("wb1", [128, 1024], F32, kind="ExternalInput")
    wb2 = nc.dram_tensor("wb2", [128, 128], F32, kind="ExternalInput")
    wt0 = nc.dram_tensor("wt0", [128, NCH * 512], F32, kind="ExternalInput")
    wt1 = nc.dram_tensor("wt1", [128, 1024], F32, kind="ExternalInput")
    wt2 = nc.dram_tensor("wt2", [128, 2], F32, kind="ExternalInput")
    bb0 = nc.dram_tensor("bb0", [128, 4], F32, kind="ExternalInput")
    bb1 = nc.dram_tensor("bb1", [128, 2], F32, kind="ExternalInput")
    bb2 = nc.dram_tensor("bb2", [64, 1], F32, kind="ExternalInput")
    tb0 = nc.dram_tensor("tb0", [128, 4], F32, kind="ExternalInput")
    tb1 = nc.dram_tensor("tb1", [128, 2], F32, kind="ExternalInput")
    tb2 = nc.dram_tensor("tb2", [1, 1], F32, kind="ExternalInput")
    y = nc.dram_tensor("y", [1, Bc], F32, kind="ExternalOutput")

    with tile.TileContext(nc) as tc:
        with (
            tc.tile_pool(name="const", bufs=1) as cpool,
            tc.tile_pool(name="acts", bufs=1) as apool,
            tc.tile_pool(name="stage", bufs=3) as spool,
            tc.tile_pool(name="mm", bufs=2, space="PSUM") as mmpool,
            tc.tile_pool(name="tp", bufs=4, space="PSUM") as tppool,
        ):
            ident = cpool.tile([128, 128], F32)
            make_identity(nc, ident[:])
            ident16 = cpool.tile([128, 128], mybir.dt.float16, tag="id16")
            make_identity(nc, ident16[:])

            def load(dram, shape, dtype=F32):
                t = cpool.tile(shape, dtype, tag=dram.name)
                nc.sync.dma_start(out=t[:], in_=dram.ap())
                return t

            idx_sb = load(idx, [128, T * NT * L], I32)
            xdt_sb = load(xdt, [128, Bc])
            wb0_sb = load(wb0, [128, 512])
            wb1_sb = load(wb1, [128, 1024])
            wb2_sb = load(wb2, [128, 128])
            wt0_sb = load(wt0, [128, NCH * 512])
            wt1_sb = load(wt1, [128, 1024])
            wt2_sb = load(wt2, [128, 2])
            bb0_sb = load(bb0, [128, 4])
            bb1_sb = load(bb1, [128, 2])
            bb2_sb = load(bb2, [64, 1])
            tb0_sb = load(tb0, [128, 4])
            tb1_sb = load(tb1, [128, 2])
            tb2_sb = load(tb2, [1, 1])

            featT = apool.tile([128, NCH * Bc], F32)
            if pad_rows:
                nc.vector.memset(featT[128 - pad_rows:, (NCH - 1) * Bc:], 0.0)

            # ---------------- bottom MLP ----------------
            h0 = apool.tile([128, 4 * Bc], F32)
            for o in range(4):
                ps = mmpool.tile([128, 512], F32)
                nc.tensor.matmul(
                    out=ps[:, :Bc], lhsT=wb0_sb[:, o * 128:(o + 1) * 128],
                    rhs=xdt_sb[:], start=True, stop=True)
                nc.scalar.activation(
                    out=h0[:, o * Bc:(o + 1) * Bc], in_=ps[:, :Bc],
                    func=AF.Relu, bias=bb0_sb[:, o:o + 1])
            h1 = apool.tile([128, 2 * Bc], F32)
            for o in range(2):
                ps = mmpool.tile([128, 512], F32)
                for k in range(4):
                    nc.tensor.matmul(
                        out=ps[:, :Bc],
                        lhsT=wb1_sb[:, k * 256 + o * 128:k * 256 + o * 128 + 128],
                        rhs=h0[:, k * Bc:(k + 1) * Bc],
                        start=(k == 0), stop=(k == 3))
                nc.scalar.activation(
                    out=h1[:, o * Bc:(o + 1) * Bc], in_=ps[:, :Bc],
                    func=AF.Relu, bias=bb1_sb[:, o:o + 1])
            ps = mmpool.tile([128, 512], F32)
            for k in range(2):
                nc.tensor.matmul(
                    out=ps[:64, :Bc], lhsT=wb2_sb[:, k * 64:(k + 1) * 64],
                    rhs=h1[:, k * Bc:(k + 1) * Bc],
                    start=(k == 0), stop=(k == 1))
            nc.scalar.activation(
                out=featT[0:64, 0:Bc], in_=ps[:64, :Bc],
                func=AF.Relu, bias=bb2_sb[:, 0:1])

            # ---------------- embedding gather + pool ----------------
            for t in range(T):
                for j in range(NT):
                    st = spool.tile([128, L, D], mybir.dt.float16, tag="stage")
                    cb = t * NT * L + j * L
                    for l in range(L):
                        nc.gpsimd.indirect_dma_start(
                            out=st[:, l, :],
                            out_offset=None,
                            in_=tables.ap(),
                            in_offset=bass.IndirectOffsetOnAxis(
                                ap=idx_sb[:, cb + l:cb + l + 1], axis=0),
                        )
                    # add-tree over the 20 lookups, in place
                    nc.vector.tensor_add(
                        out=st[:, 0:10, :], in0=st[:, 0:10, :],
                        in1=st[:, 10:20, :])
                    nc.vector.tensor_add(
                        out=st[:, 0:5, :], in0=st[:, 0:5, :], in1=st[:, 5:10, :])
                    nc.vector.tensor_add(
                        out=st[:, 0:2, :], in0=st[:, 0:2, :], in1=st[:, 2:4, :])
                    nc.vector.tensor_add(
                        out=st[:, 0:1, :], in0=st[:, 0:1, :], in1=st[:, 1:2, :])
                    nc.vector.tensor_add(
                        out=st[:, 0:1, :], in0=st[:, 0:1, :], in1=st[:, 4:5, :])
                    pst = tppool.tile([64, 128], mybir.dt.float16, tag="tp")
                    nc.tensor.transpose(
                        out=pst[:], in_=st[:, 0, :], identity=ident16[:])
                    c = (t + 1) // 2
                    off = 64 * ((t + 1) % 2)
                    nc.scalar.copy(
                        out=featT[off:off + 64,
                                  c * Bc + j * 128:c * Bc + (j + 1) * 128],
                        in_=pst[:])

            # ---------------- top MLP ----------------
            z0 = apool.tile([128, 4 * Bc], F32)
            for o in range(4):
                ps = mmpool.tile([128, 512], F32)
                for k in range(NCH):
                    nc.tensor.matmul(
                        out=ps[:, :Bc],
                        lhsT=wt0_sb[:, k * 512 + o * 128:k * 512 + o * 128 + 128],
                        rhs=featT[:, k * Bc:(k + 1) * Bc],
                        start=(k == 0), stop=(k == NCH - 1))
                nc.scalar.activation(
                    out=z0[:, o * Bc:(o + 1) * Bc], in_=ps[:, :Bc],
                    func=AF.Relu, bias=tb0_sb[:, o:o + 1])
            z1 = apool.tile([128, 2 * Bc], F32)
            for o in range(2):
                ps = mmpool.tile([128, 512], F32)
                for k in range(4):
                    nc.tensor.matmul(
                        out=ps[:, :Bc],
                        lhsT=wt1_sb[:, k * 256 + o * 128:k * 256 + o * 128 + 128],
                        rhs=z0[:, k * Bc:(k + 1) * Bc],
                        start=(k == 0), stop=(k == 3))
                nc.scalar.activation(
                    out=z1[:, o * Bc:(o + 1) * Bc], in_=ps[:, :Bc],
                    func=AF.Relu, bias=tb1_sb[:, o:o + 1])
            ps = mmpool.tile([128, 512], F32)
            for k in range(2):
                nc.tensor.matmul(
                    out=ps[0:1, :Bc], lhsT=wt2_sb[:, k:k + 1],
                    rhs=z1[:, k * Bc:(k + 1) * Bc],
                    start=(k == 0), stop=(k == 1))
            ysb = apool.tile([1, Bc], F32)
            nc.scalar.activation(
                out=ysb[:], in_=ps[0:1, :Bc],
                func=AF.Sigmoid, bias=tb2_sb[0:1, 0:1])
            nc.sync.dma_start(out=y.ap(), in_=ysb[:])

    nc.compile()
    return nc


def pack_weights(inp, T=T, D=D):
    NCH = (T + 2) // 2
    f32 = np.float32

    def kchunks(wT, K, M):
        return np.ascontiguousarray(
            wT.reshape(K // 128, 128, M).transpose(1, 0, 2).reshape(128, -1)
        ).astype(f32, copy=False)

    wb0 = np.zeros((128, 512), f32)
    wb0[:DENSE] = inp["bw0"].T
    wb1 = kchunks(np.ascontiguousarray(inp["bw1"].T), 512, 256)
    wb2 = kchunks(np.ascontiguousarray(inp["bw2"].T), 256, 64)
    feat_in = D * (1 + T)
    wt0p = np.zeros((NCH * 128, 512), f32)
    wt0p[:feat_in] = inp["tw0"].T
    wt0 = kchunks(wt0p, NCH * 128, 512)
    wt1 = kchunks(np.ascontiguousarray(inp["tw1"].T), 512, 256)
    wt2 = kchunks(np.ascontiguousarray(inp["tw2"].T), 256, 1)
    return dict(
        wb0=wb0, wb1=wb1, wb2=wb2, wt0=wt0, wt1=wt1, wt2=wt2,
        bb0=np.ascontiguousarray(inp["bb0"].reshape(4, 128).T).astype(f32),
        bb1=np.ascontiguousarray(inp["bb1"].reshape(2, 128).T).astype(f32),
        bb2=inp["bb2"].reshape(64, 1).astype(f32),
        tb0=np.ascontiguousarray(inp["tb0"].reshape(4, 128).T).astype(f32),
        tb1=np.ascontiguousarray(inp["tb1"].reshape(2, 128).T).astype(f32),
        tb2=inp["tb2"].reshape(1, 1).astype(f32),
    )


def pack_core(x_dense, x_indices, c, Bc, NT, T=T, V=V, L=L):
    sl = slice(c * Bc, (c + 1) * Bc)
    xdt = np.zeros((128, Bc), np.float32)
    xdt[:DENSE] = x_dense[sl].T
    idx = x_indices[:, sl, :].astype(np.int32)           # [T, Bc, L]
    idx += (np.arange(T, dtype=np.int32) * V)[:, None, None]
    idxp = np.ascontiguousarray(
        idx.reshape(T, NT, 128, L).transpose(2, 0, 1, 3).reshape(128, T * NT * L)
    )
    return xdt, idxp


_NC_CACHE = {}


def _get_nc():
    if "nc" not in _NC_CACHE:
        _NC_CACHE["nc"] = build_bass()
    return _NC_CACHE["nc"]


def run(inputs, trace=False, **run_kwargs):
    nc = _get_nc()
    NT = 4
    Bc = NT * 128
    shared = pack_weights(inputs)
    tables_flat = np.ascontiguousarray(
        np.asarray(inputs["tables"], dtype=np.float32).reshape(T * V, D)
    ).astype(np.float16)
    x_dense = np.asarray(inputs["x_dense"], dtype=np.float32)
    x_indices = np.asarray(inputs["x_indices"])
    in_maps = []
    for c in range(NCORES):
        xdt, idxp = pack_core(x_dense, x_indices, c, Bc, NT)
        m = dict(shared)
        m["tables"] = tables_flat
        m["xdt"] = xdt
        m["idx"] = idxp
        in_maps.append(m)
    res = run_bass_kernel_spmd(
        nc, in_maps, core_ids=list(range(NCORES)), trace=trace, **run_kwargs)
    yv = np.concatenate([res.results[c]["y"][0] for c in range(NCORES)])
    return yv.reshape(B, 1).astype(np.float32), res


def kernel(**inputs):
    return run(inputs)[0]



# revision 12
# speedup vs baseline: 8.8906x; 8.8906x over previous
"""DLRM (embedding_lookup) Trainium2 Bass kernel.

Strategy: pure data parallelism over the batch. Each of the 8 NeuronCores
holds all 26 embedding tables (replicated in its HBM, flattened to one
[26*200000, 64] tensor so indirect DMA sees base offset 0) and processes
a 512-sample slice of the 4096 batch end-to-end:

  bottom MLP (13->512->256->64) feature-major on PE/ACT
  embedding gathers: per (table, sample-tile, lookup) one
      gpsimd.indirect_dma_start with a [128,1] int32 offset column
      (host pre-biases indices by t*V) fetching 128 rows, one per
      partition, into staging [128, 20, 64]; a DVE add-tree pools the
      20 lookups; PE transposes pooled [128,64] -> [64,128] into the
      feature-major feat matrix [1792, 512] (64 zero pad rows)
  top MLP (1728->512->256->1) + sigmoid on PE/ACT

No collectives; host shards inputs / concatenates outputs. This is
perfectly load balanced (26 tables over 8 cores would be 4-vs-3
imbalanced table-wise) and needs no all-to-all.
"""

import numpy as np

import concourse.bass as bass
import concourse.mybir as mybir
import concourse.tile as tile
from concourse import bacc
from concourse.bass_utils import run_bass_kernel_spmd
from concourse.masks import make_identity

F32 = mybir.dt.float32
I32 = mybir.dt.int32
AF = mybir.ActivationFunctionType

B = 4096
T = 26
V = 200000
L = 20
D = 64
DENSE = 13
NCORES = 8


def build_bass(T=T, V=V, NT=4, L=L, D=D):
    Bc = NT * 128
    NCH = (T + 2) // 2
    pad_rows = NCH * 128 - D * (T + 1)
    assert Bc <= 512

    nc = bacc.Bacc(
        "TRN2", target_bir_lowering=False, debug=False,
        enable_asserts=False, num_devices=1,
    )

    tables = nc.dram_tensor("tables", [T * V, D], mybir.dt.float16, kind="ExternalInput")
    idx = nc.dram_tensor("idx", [128, T * NT * L], I32, kind="ExternalInput")
    xdt = nc.dram_tensor("xdt", [128, Bc], F32, kind="ExternalInput")
    wb0 = nc.dram_tensor("wb0", [128, 512], F32, kind="ExternalInput")
    wb1 = nc.dram_tensor("wb1", [128, 1024], F32, kind="ExternalInput")
    wb2 = nc.dram_tensor("wb2", [128, 128], F32, kind="ExternalInput")
    wt0 = nc.dram_tensor("wt0", [128, NCH * 512], F32, kind="ExternalInput")
    wt1 = nc.dram_tensor("wt1", [128, 1024], F32, kind="ExternalInput")
    wt2 = nc.dram_tensor("wt2", [128, 2], F32, kind="ExternalInput")
    bb0 = nc.dram_tensor("bb0", [128, 4], F32, kind="ExternalInput")
    bb1 = nc.dram_tensor("bb1", [128, 2], F32, kind="ExternalInput")
    bb2 = nc.dram_tensor("bb2", [64, 1], F32, kind="ExternalInput")
    tb0 = nc.dram_tensor("tb0", [128, 4], F32, kind="ExternalInput")
    tb1 = nc.dram_tensor("tb1", [128, 2], F32, kind="ExternalInput")
    tb2 = nc.dram_tensor("tb2", [1, 1], F32, kind="ExternalInput")
    y = nc.dram_tensor("y", [1, Bc], F32, kind="ExternalOutput")

    with tile.TileContext(nc) as tc:
        with (
            tc.tile_pool(name="const", bufs=1) as cpool,
            tc.tile_pool(name="acts", bufs=1) as apool,
            tc.tile_pool(name="stage", bufs=3) as spool,
            tc.tile_pool(name="mm", bufs=2, space="PSUM") as mmpool,
            tc.tile_pool(name="tp", bufs=4, space="PSUM") as tppool,
        ):
            ident = cpool.tile([128, 128], F32)
            make_identity(nc, ident[:])
            ident16 = cpool.tile([128, 128], mybir.dt.float16, tag="id16")
            make_identity(nc, ident16[:])

            def load(dram, shape, dtype=F32):
                t = cpool.tile(shape, dtype, tag=dram.name)
                nc.sync.dma_start(out=t[:], in_=dram.ap())
                return t

            idx_sb = load(idx, [128, T * NT * L], I32)
            xdt_sb = load(xdt, [128, Bc])
            wb0_sb = load(wb0, [128, 512])
            wb1_sb = load(wb1, [128, 1024])
            wb2_sb = load(wb2, [128, 128])
            wt0_sb = load(wt0, [128, NCH * 512])
            wt1_sb = load(wt1, [128, 1024])
            wt2_sb = load(wt2, [128, 2])
            bb0_sb = load(bb0, [128, 4])
            bb1_sb = load(bb1, [128, 2])
            bb2_sb = load(bb2, [64, 1])
            tb0_sb = load(tb0, [128, 4])
            tb1_sb = load(tb1, [128, 2])
            tb2_sb = load(tb2, [1, 1])

            featT = apool.tile([128, NCH * Bc], F32)
            if pad_rows:
                nc.vector.memset(featT[128 - pad_rows:, (NCH - 1) * Bc:], 0.0)

            # ---------------- bottom MLP ----------------
            h0 = apool.tile([128, 4 * Bc], F32)
            for o in range(4):
                ps = mmpool.tile([128, 512], F32)
                nc.tensor.matmul(
                    out=ps[:, :Bc], lhsT=wb0_sb[:, o * 128:(o + 1) * 128],
                    rhs=xdt_sb[:], start=True, stop=True)
                nc.scalar.activation(
                    out=h0[:, o * Bc:(o + 1) * Bc], in_=ps[:, :Bc],
                    func=AF.Relu, bias=bb0_sb[:, o:o + 1])
            h1 = apool.tile([128, 2 * Bc], F32)
            for o in range(2):
                ps = mmpool.tile([128, 512], F32)
                for k in range(4):
                    nc.tensor.matmul(
                        out=ps[:, :Bc],
                        lhsT=wb1_sb[:, k * 256 + o * 128:k * 256 + o * 128 + 128],
                        rhs=h0[:, k * Bc:(k + 1) * Bc],
                        start=(k == 0), stop=(k == 3))
                nc.scalar.activation(
                    out=h1[:, o * Bc:(o + 1) * Bc], in_=ps[:, :Bc],
                    func=AF.Relu, bias=bb1_sb[:, o:o + 1])
            ps = mmpool.tile([128, 512], F32)
            for k in range(2):
                nc.tensor.matmul(
                    out=ps[:64, :Bc], lhsT=wb2_sb[:, k * 64:(k + 1) * 64],
                    rhs=h1[:, k * Bc:(k + 1) * Bc],
                    start=(k == 0), stop=(k == 1))
            nc.scalar.activation(
                out=featT[0:64, 0:Bc], in_=ps[:64, :Bc],
                func=AF.Relu, bias=bb2_sb[:, 0:1])

            # ---------------- embedding gather + pool ----------------
            for t in range(T):
                for j in range(NT):
                    # one batched gather per (table, tile) into a FLAT 2D
                    # staging tile: identical descriptor stream to the 3D
                    # variant, but canonical 2D APs end to end.
                    st = spool.tile([128, L * D], mybir.dt.float16, tag="stage")
                    cb = t * NT * L + j * L
                    nc.gpsimd.indirect_dma_start(
                        out=st[:],
                        out_offset=None,
                        in_=tables.ap(),
                        in_offset=bass.IndirectOffsetOnAxis(
                            ap=idx_sb[:, cb:cb + L], axis=0),
                    )
                    # add-tree over the 20 lookups (64-col rows), in place
                    nc.vector.tensor_add(
                        out=st[:, 0:640], in0=st[:, 0:640], in1=st[:, 640:1280])
                    nc.vector.tensor_add(
                        out=st[:, 0:320], in0=st[:, 0:320], in1=st[:, 320:640])
                    nc.vector.tensor_add(
                        out=st[:, 0:128], in0=st[:, 0:128], in1=st[:, 128:256])
                    nc.vector.tensor_add(
                        out=st[:, 0:64], in0=st[:, 0:64], in1=st[:, 64:128])
                    nc.vector.tensor_add(
                        out=st[:, 0:64], in0=st[:, 0:64], in1=st[:, 256:320])
                    pst = tppool.tile([64, 128], mybir.dt.float16, tag="tp")
                    nc.tensor.transpose(
                        out=pst[:], in_=st[:, 0:64], identity=ident16[:])
                    c = (t + 1) // 2
                    off = 64 * ((t + 1) % 2)
                    nc.scalar.copy(
                        out=featT[off:off + 64,
                                  c * Bc + j * 128:c * Bc + (j + 1) * 128],
                        in_=pst[:])

            # ---------------- top MLP ----------------
            z0 = apool.tile([128, 4 * Bc], F32)
            for o in range(4):
                ps = mmpool.tile([128, 512], F32)
                for k in range(NCH):
                    nc.tensor.matmul(
                        out=ps[:, :Bc],
                        lhsT=wt0_sb[:, k * 512 + o * 128:k * 512 + o * 128 + 128],
                        rhs=featT[:, k * Bc:(k + 1) * Bc],
                        start=(k == 0), stop=(k == NCH - 1))
                nc.scalar.activation(
                    out=z0[:, o * Bc:(o + 1) * Bc], in_=ps[:, :Bc],
                    func=AF.Relu, bias=tb0_sb[:, o:o + 1])
            z1 = apool.tile([128, 2 * Bc], F32)
            for o in range(2):
                ps = mmpool.tile([128, 512], F32)
                for k in range(4):
                    nc.tensor.matmul(
                        out=ps[:, :Bc],
                        lhsT=wt1_sb[:, k * 256 + o * 128:k * 256 + o * 128 + 128],
                        rhs=z0[:, k * Bc:(k + 1) * Bc],
                        start=(k == 0), stop=(k == 3))
                nc.scalar.activation(
                    out=z1[:, o * Bc:(o + 1) * Bc], in_=ps[:, :Bc],
                    func=AF.Relu, bias=tb1_sb[:, o:o + 1])
            ps = mmpool.tile([128, 512], F32)
            for k in range(2):
                nc.tensor.matmul(
                    out=ps[0:1, :Bc], lhsT=wt2_sb[:, k:k + 1],
                    rhs=z1[:, k * Bc:(k + 1) * Bc],
                    start=(k == 0), stop=(k == 1))
            ysb = apool.tile([1, Bc], F32)
            nc.scalar.activation(
                out=ysb[:], in_=ps[0:1, :Bc],
                func=AF.Sigmoid, bias=tb2_sb[0:1, 0:1])
            nc.sync.dma_start(out=y.ap(), in_=ysb[:])

    nc.compile()
    return nc


def pack_weights(inp, T=T, D=D):
    NCH = (T + 2) // 2
    f32 = np.float32

    def kchunks(wT, K, M):
        return np.ascontiguousarray(
            wT.reshape(K // 128, 128, M).transpose(1, 0, 2).reshape(128, -1)
        ).astype(f32, copy=False)

    wb0 = np.zeros((128, 512), f32)
    wb0[:DENSE] = inp["bw0"].T
    wb1 = kchunks(np.ascontiguousarray(inp["bw1"].T), 512, 256)
    wb2 = kchunks(np.ascontiguousarray(inp["bw2"].T), 256, 64)
    feat_in = D * (1 + T)
    wt0p = np.zeros((NCH * 128, 512), f32)
    wt0p[:feat_in] = inp["tw0"].T
    wt0 = kchunks(wt0p, NCH * 128, 512)
    wt1 = kchunks(np.ascontiguousarray(inp["tw1"].T), 512, 256)
    wt2 = kchunks(np.ascontiguousarray(inp["tw2"].T), 256, 1)
    return dict(
        wb0=wb0, wb1=wb1, wb2=wb2, wt0=wt0, wt1=wt1, wt2=wt2,
        bb0=np.ascontiguousarray(inp["bb0"].reshape(4, 128).T).astype(f32),
        bb1=np.ascontiguousarray(inp["bb1"].reshape(2, 128).T).astype(f32),
        bb2=inp["bb2"].reshape(64, 1).astype(f32),
        tb0=np.ascontiguousarray(inp["tb0"].reshape(4, 128).T).astype(f32),
        tb1=np.ascontiguousarray(inp["tb1"].reshape(2, 128).T).astype(f32),
        tb2=inp["tb2"].reshape(1, 1).astype(f32),
    )


def pack_core(x_dense, x_indices, c, Bc, NT, T=T, V=V, L=L):
    sl = slice(c * Bc, (c + 1) * Bc)
    xdt = np.zeros((128, Bc), np.float32)
    xdt[:DENSE] = x_dense[sl].T
    idx = x_indices[:, sl, :].astype(np.int32)           # [T, Bc, L]
    idx += (np.arange(T, dtype=np.int32) * V)[:, None, None]
    idxp = np.ascontiguousarray(
        idx.reshape(T, NT, 128, L).transpose(2, 0, 1, 3).reshape(128, T * NT * L)
    )
    return xdt, idxp


_NC_CACHE = {}


def _get_nc():
    if "nc" not in _NC_CACHE:
        _NC_CACHE["nc"] = build_bass()
    return _NC_CACHE["nc"]


def run(inputs, trace=False, **run_kwargs):
    nc = _get_nc()
    NT = 4
    Bc = NT * 128
    shared = pack_weights(inputs)
    tables_flat = np.ascontiguousarray(
        np.asarray(inputs["tables"], dtype=np.float32).reshape(T * V, D)
    ).astype(np.float16)
    x_dense = np.asarray(inputs["x_dense"], dtype=np.float32)
    x_indices = np.asarray(inputs["x_indices"])
    in_maps = []
    for c in range(NCORES):
        xdt, idxp = pack_core(x_dense, x_indices, c, Bc, NT)
        m = dict(shared)
        m["tables"] = tables_flat
        m["xdt"] = xdt
        m["idx"] = idxp
        in_maps.append(m)
    res = run_bass_kernel_spmd(
        nc, in_maps, core_ids=list(range(NCORES)), trace=trace, **run_kwargs)
    yv = np.concatenate([res.results[c]["y"][0] for c in range(NCORES)])
    return yv.reshape(B, 1).astype(np.float32), res


def kernel(**inputs):
    return run(inputs)[0]

